# revision 20
# baseline (speedup 1.0000x reference)
"""Trainium2 Bass kernel for nn_GroupConvolutionLayer2d.

Computation (see reference):
  xn = (x - mean(x, -1)) / (std(x, -1) + 1e-7)          # per-row normalize
  lm = circular_conv(lm_raw, gauss_filt(sigma=0.1))      # along last axis
  y[b, i, j] = sum_n lm[i, j, n] * xn[b, n]              # [16384, 32, 32]

Strategy: data-parallel over batch across 8 NeuronCores (2048 rows each).

Design notes (on top of the v2 PE-stationary design):
  * Normalization applied AFTER the matmul (linear):
      y[b,p] = inv_b * (z[b,p] - mu_b * s[p]),  z = x @ lmT,
      s[p] = column sums of lmroll (filter sums to 1).
  * All inputs are host-packed into SBUF-image layouts (pure layout
    transforms + bf16 cast) so every DMA is contiguous-per-partition;
    conv-critical bytes (cb, then lmroll chunks) are issued first at
    ~256KB granularity - the ~0.6us per-issue sequencer cost then paces
    arrival smoothly just above HBM rate.  Coarser DMAs make arrival
    lumpy, starve the PE mid-conv and reset the HAM warm-up (measured:
    the whole conv then runs at 1.2GHz).
  * PE warmup dummies bridge from body start until the first conv
    chunk's DMA lands (~11us) so the HAM clock boost (1.2->2.4GHz)
    engages before real work; tile-0 main matmuls interleave into the
    conv phase; no trailing dummies (they only delayed the drain).
  * y is staged in one [128, 16, 1024] SBUF tile and shipped in batched
    DMAs; the LAST b-tile runs its matmuls column-quarter-major with a
    separate PSUM tile per quarter (a shared tile adds a WAR stall per
    quarter) so the copy/subtract/DMA tail drains under the remaining
    matmuls (~1.6us tail instead of ~4.4us).  Output quarter-DMAs stay
    at 512B/partition: half-DMAs would sit below the SDMA line size and
    the read-modify-write races the adjacent half on the shared HBM
    line (observed as a nondeterministic wrong quarter).
"""

import os
import sys

import numpy as np

for _p in ("/opt/trn_rl_repo",):
    if _p not in sys.path and os.path.isdir(_p):
        sys.path.insert(0, _p)

import ml_dtypes  # noqa: E402

import concourse.bass as bass  # noqa: E402
import concourse.mybir as mybir  # noqa: E402
import concourse.tile as tile  # noqa: E402
from concourse import bacc  # noqa: E402
from concourse.bass_utils import run_bass_kernel_spmd  # noqa: E402

N_CORES = 8
B_FULL = 16384
BS = B_FULL // N_CORES  # 2048 rows per core
NIN = 1024
P = 1024  # 32*32 output grid, flattened
NT = BS // 128  # 16 b-tiles per core
KT = NIN // 128  # 8 contraction chunks
FILT = 33
PAD = FILT // 2  # 16
SIGMA0 = 0.1
EPS = 1e-7

CBW = 256  # cb columns in the packed lm image (2 blocks of 128)
LMW = CBW + KT * NIN  # total packed-lm image columns (8448)

BF16 = ml_dtypes.bfloat16


def _gauss_filt() -> np.ndarray:
    t = (np.arange(FILT, dtype=np.float32) - FILT // 2) * np.float32(2.0 / FILT)
    k = np.exp(-0.5 * np.square(t / np.float32(SIGMA0)))
    return (k / k.sum()).astype(np.float32)


def _cb_blocks() -> np.ndarray:
    """Stationary blocks for the rolled banded-circulant conv matmul.

    lm[p, n] = sum_t filt[t] * lm_raw[p, (n + t - 16) % 1024]
    With lmroll[m'] = lm_rawT[(m' - 16) % 1024] the weight linking rolled
    row m' to output n is filt[m' - n], m' - n in [0, 32].  For output
    chunk ni the contributing m' live in chunks ni (B0) and ni+1 (B1):
      B0[mh, nh] = filt[mh - nh]        for 0 <= mh - nh <= 32
      B1[mh, nh] = filt[mh + 128 - nh]  for 0 <= mh + 128 - nh <= 32
    """
    filt = _gauss_filt()
    mh = np.arange(128)[:, None]
    nh = np.arange(128)[None, :]
    out = np.zeros((2, 128, 128), dtype=np.float32)
    d0 = mh - nh
    out[0] = np.where((d0 >= 0) & (d0 < FILT), filt[np.clip(d0, 0, FILT - 1)], 0.0)
    d1 = mh + 128 - nh
    out[1] = np.where((d1 >= 0) & (d1 < FILT), filt[np.clip(d1, 0, FILT - 1)], 0.0)
    return out


def _build_kernel_body(tc: "tile.TileContext", y_ap, xt_ap, xrow_ap, lmimg_ap):
    nc = tc.nc
    f32 = mybir.dt.float32
    bf16 = mybir.dt.bfloat16

    with (
        tc.tile_pool(name="const", bufs=1) as const_pool,
        tc.tile_pool(name="lm", bufs=1) as lm_pool,
        tc.tile_pool(name="stat", bufs=16) as stat_pool,
        tc.tile_pool(name="t1p", bufs=3) as t1_pool,
        tc.tile_pool(name="pz0p", bufs=1, space="PSUM") as pz0_pool,
        tc.tile_pool(name="pmm", bufs=3, space="PSUM") as pmm_pool,
    ):
        # ---- constants / big SBUF staging ----
        ident = const_pool.tile([128, 128], bf16)
        ones_col = const_pool.tile([128, 1], bf16)
        ones_row = const_pool.tile([1, 128], bf16)
        s_row = const_pool.tile([1, P], bf16)
        usum = const_pool.tile([128, P], bf16)
        s_bcast = const_pool.tile([128, P], bf16)

        lmimg_sb = lm_pool.tile([128, LMW], bf16)
        lmT_sb = lm_pool.tile([128, KT, P], bf16)
        xt_sb = lm_pool.tile([128, NT, KT, 128], bf16)
        xrow_sb = lm_pool.tile([128, NT, NIN], bf16)
        ybig = lm_pool.tile([128, NT, P], bf16)

        def cbv(s):
            return lmimg_sb[:, s * 128 : (s + 1) * 128]

        def lmr(mi):
            return lmimg_sb[:, CBW + mi * NIN : CBW + (mi + 1) * NIN]

        # ident for the PE warmup: vector is the first engine free after the
        # framework's const-ap barrier, so the warmup can start ~immediately.
        nc.vector.memset(ident, 1.0)
        nc.gpsimd.memset(ones_col, 1.0)
        nc.gpsimd.memset(ones_row, 1.0)
        sqwarm = const_pool.tile([1, 1], f32)
        nc.gpsimd.memset(sqwarm, 1.0)

        # dummy Sqrt preloads the scalar engine's activation table during
        # the idle prologue (otherwise a 1.3us ACT_TABLE_LOAD lands mid-run
        # right before the first real sqrt)
        nc.scalar.activation(
            out=sqwarm, in_=sqwarm, func=mybir.ActivationFunctionType.Sqrt
        )

        # ---- input DMA issue order (single sync HW queue, FIFO).  ~256KB
        # granularity: the ~0.6us per-issue sequencer cost then paces the
        # stream at ~430GB/s (just above HBM rate) with SMOOTH arrival -
        # large consolidated DMAs make arrival lumpy, starve the PE during
        # the conv phase, and keep resetting the HAM warm-up (measured: the
        # whole conv ran at 1.2GHz that way).  Conv-critical chunks first.
        def lm_dma(a, b):
            nc.sync.dma_start(out=lmimg_sb[:, a:b], in_=lmimg_ap[:, a:b])

        lm_dma(0, CBW)  # cb alone first: tiny, lets conv LDWEIGHTS pre-run
        for mi in range(0, 4):
            lm_dma(CBW + mi * NIN, CBW + (mi + 1) * NIN)
        nc.sync.dma_start(out=xt_sb[:, 0:1], in_=xt_ap[:, 0:1])
        for mi in range(4, KT):
            lm_dma(CBW + mi * NIN, CBW + (mi + 1) * NIN)
        nc.sync.dma_start(out=xt_sb[:, 1:2], in_=xt_ap[:, 1:2])
        nc.sync.dma_start(out=xt_sb[:, 2:4], in_=xt_ap[:, 2:4])
        nc.sync.dma_start(out=xrow_sb[:, 0:2], in_=xrow_ap[:, 0:2])
        nc.sync.dma_start(out=xt_sb[:, 4:6], in_=xt_ap[:, 4:6])
        nc.sync.dma_start(out=xrow_sb[:, 2:4], in_=xrow_ap[:, 2:4])
        nc.sync.dma_start(out=xt_sb[:, 6:9], in_=xt_ap[:, 6:9])
        nc.sync.dma_start(out=xrow_sb[:, 4:7], in_=xrow_ap[:, 4:7])
        nc.sync.dma_start(out=xt_sb[:, 9:12], in_=xt_ap[:, 9:12])
        nc.sync.dma_start(out=xrow_sb[:, 7:10], in_=xrow_ap[:, 7:10])
        nc.sync.dma_start(out=xt_sb[:, 12:16], in_=xt_ap[:, 12:16])
        nc.sync.dma_start(out=xrow_sb[:, 10:16], in_=xrow_ap[:, 10:16])

        # ---- PE warm-up: dummy matmuls bridge from body start (~7.2us)
        # until the first conv chunk's DMA lands (~11us); the HAM flip
        # (~3.5us of sustained busy) engages before real work starts.
        # (N=1 const-ap dummies were tried: they issue at 25ns each and do
        # NOT register as PE activity for the HAM - the conv then runs at
        # 1.2GHz.  N=128 ident dummies at ~107ns do warm it.)
        pw = pmm_pool.tile([128, P], f32, tag="mm")
        for _ in range(32):
            nc.tensor.matmul(
                pw[:, 0:128], lhsT=ident, rhs=ident, start=True, stop=True
            )

        def mm_tile0(pz0, ni):
            for h in range(2):
                sl = slice(h * 512, (h + 1) * 512)
                nc.tensor.matmul(
                    pz0[:, sl],
                    lhsT=xt_sb[:, 0, ni, :],
                    rhs=lmT_sb[:, ni, sl],
                    start=(ni == 0),
                    stop=(ni == KT - 1),
                )

        # ---- banded conv matmul: lmT[ni] = B0.T @ lmroll[ni] + B1.T @ lmroll[ni+1]
        pz0 = pz0_pool.tile([128, P], f32, tag="pz0")

        def conv_chunk(ni, pad=0):
            pc = pmm_pool.tile([128, P], f32, tag="mm")
            for h in range(2):
                sl = slice(h * 512, (h + 1) * 512)
                nc.tensor.matmul(
                    pc[:, sl], lhsT=cbv(0), rhs=lmr(ni)[:, sl], start=True, stop=False
                )
            # early chunks: a few dummy matmuls between the B0 and B1
            # accumulation pairs keep the PE busy across DMA-arrival
            # jitter of lmroll[ni+1] - a >0.4us gap here resets the HAM
            # busy-window and the whole conv then runs at 1.2GHz
            # (observed: +2.4us on such runs).
            for _ in range(pad):
                nc.tensor.matmul(
                    pw[:, 0:128], lhsT=ident, rhs=ident, start=True, stop=True
                )
            for h in range(2):
                sl = slice(h * 512, (h + 1) * 512)
                nc.tensor.matmul(
                    pc[:, sl],
                    lhsT=cbv(1),
                    rhs=lmr((ni + 1) % KT)[:, sl],
                    start=False,
                    stop=True,
                )
            # split the PSUM->SBUF copy across scalar+vector so lmT[ni] is
            # ready ~0.6us after the matmuls instead of ~1.1us.
            nc.scalar.copy(out=lmT_sb[:, ni, 0:512], in_=pc[:, 0:512])
            nc.vector.tensor_scalar_add(
                out=lmT_sb[:, ni, 512:1024], in0=pc[:, 512:1024], scalar1=0.0
            )

        # conv chunks interleaved with tile-0 main matmuls roughly in DMA
        # arrival order (lm chunks land ~0.65us apart, conv consumes 0.43).
        conv_chunk(0, pad=3)
        conv_chunk(1, pad=1)
        conv_chunk(2, pad=1)
        mm_tile0(pz0, 0)
        conv_chunk(3)
        mm_tile0(pz0, 1)
        conv_chunk(4)
        mm_tile0(pz0, 2)
        conv_chunk(5)
        mm_tile0(pz0, 3)
        conv_chunk(6)
        mm_tile0(pz0, 4)
        conv_chunk(7)

        # ---- s[p] = colsum of lmroll: partial sums on the vector engine,
        # partition-reduce + rank-1 broadcast on the PE (tiny work).
        nc.vector.tensor_tensor(
            out=usum, in0=lmr(0), in1=lmr(1), op=mybir.AluOpType.add
        )
        for mi in range(2, KT):
            nc.vector.tensor_tensor(
                out=usum, in0=usum, in1=lmr(mi), op=mybir.AluOpType.add
            )
        mm_tile0(pz0, 5)
        mm_tile0(pz0, 6)
        mm_tile0(pz0, 7)

        def s_block():
            # s partition-reduce + rank-1 broadcast on the PE.  Emitted
            # after tile-1's matmuls: ps waits on usum, which the (busy)
            # vector queue only finishes ~3.5us after conv7 - emitted any
            # earlier it blocks the in-order PE queue for that long.  The
            # matmuls reuse pw's already-held PSUM slot so the pc/pz pool
            # rotation is unaffected.
            ps = pw
            for h in range(2):
                sl = slice(h * 512, (h + 1) * 512)
                nc.tensor.matmul(
                    ps[0:1, sl], lhsT=ones_col, rhs=usum[:, sl], start=True, stop=True
                )
            nc.scalar.copy(out=s_row, in_=ps[0:1, :])
            psb = pw
            for h in range(2):
                sl = slice(h * 512, (h + 1) * 512)
                nc.tensor.matmul(
                    psb[:, sl], lhsT=ones_row, rhs=s_row[:, sl], start=True, stop=True
                )
            nc.vector.tensor_scalar_add(out=s_bcast, in0=psb, scalar1=0.0)

        # ---- per-row stats (vector/scalar engines; overlap the matmuls) ----
        def emit_stats(t, invs, cs):
            st = stat_pool.tile([128, 2, 6], f32, tag="st")
            nc.vector.bn_stats(out=st[:, 0, :], in_=xrow_sb[:, t, 0:512])
            nc.vector.bn_stats(out=st[:, 1, :], in_=xrow_sb[:, t, 512:1024])
            mv = stat_pool.tile([128, 2], f32, tag="mv")
            nc.vector.bn_aggr(out=mv, in_=st)
            sd = stat_pool.tile([128, 1], f32, tag="sd")
            nc.scalar.activation(
                out=sd, in_=mv[:, 1:2], func=mybir.ActivationFunctionType.Sqrt
            )
            # EPS=1e-7 on sd~1.0 is far below bf16 noise; skip the add
            inv = stat_pool.tile([128, 1], f32, tag="inv")
            nc.vector.reciprocal(out=inv, in_=sd)
            c = stat_pool.tile([128, 1], f32, tag="c")
            nc.gpsimd.tensor_tensor(
                out=c, in0=mv[:, 0:1], in1=inv, op=mybir.AluOpType.mult
            )
            invs.append(inv)
            cs.append(c)

        invs = []
        cs = []
        emit_stats(0, invs, cs)
        emit_stats(1, invs, cs)

        # ---- main matmul: z_t = x_t @ lmT; y_t = inv*(z_t - mu*s) ----
        t1s = {}

        def emit_t1(t):
            # t1 = (mu*inv) * s does not depend on z: compute it during the
            # matmuls (scalar engine activation with per-partition scale)
            t1 = t1_pool.tile([128, P], bf16)
            nc.scalar.activation(
                out=t1,
                in_=s_bcast,
                func=mybir.ActivationFunctionType.Copy,
                scale=cs[t],
            )
            t1s[t] = t1

        def post_tile(t, pz):
            # y = z*inv (scalar, fused into the PSUM->SBUF copy) - t1
            nc.scalar.activation(
                out=ybig[:, t, :],
                in_=pz,
                func=mybir.ActivationFunctionType.Copy,
                scale=invs[t],
            )
            nc.vector.tensor_tensor(
                out=ybig[:, t, :],
                in0=ybig[:, t, :],
                in1=t1s[t],
                op=mybir.AluOpType.subtract,
            )

        for t in range(NT):
            if t + 2 < NT:
                emit_stats(t + 2, invs, cs)
            if t >= 2:
                emit_t1(t)
            if t == 0:
                # tile-0's matmuls were interleaved into the conv phase;
                # its post-processing is deferred to t==1 (after s_block)
                continue
            if t < NT - 1:
                # every 4th tile borrows pz0's (long-free) PSUM slot so the
                # effective rotation depth is 4 - removes the periodic
                # ~53ns pool-WAR stall every 3rd tile.
                if t % 4 == 0:
                    pz = pz0_pool.tile([128, P], f32, tag="pz0")
                else:
                    pz = pmm_pool.tile([128, P], f32, tag="mm")
                for ni in range(KT):
                    for h in range(2):
                        sl = slice(h * 512, (h + 1) * 512)
                        nc.tensor.matmul(
                            pz[:, sl],
                            lhsT=xt_sb[:, t, ni, :],
                            rhs=lmT_sb[:, ni, sl],
                            start=(ni == 0),
                            stop=(ni == KT - 1),
                        )
                if t == 1:
                    s_block()
                    emit_t1(0)
                    emit_t1(1)
                    post_tile(0, pz0)
                    post_tile(1, pz)
                else:
                    post_tile(t, pz)
            else:
                # last tile: column-quarter-major so each quarter's
                # copy/subtract/DMA drains under the remaining matmuls.
                # Separate PSUM tiles per quarter - a shared tile adds a
                # WAR dep from quarter q+1's matmuls to quarter q's copy
                # (~0.7us stall each).
                for q in range(4):
                    sl = slice(q * 256, (q + 1) * 256)
                    pzq = pmm_pool.tile([128, 256], f32, tag="mm")
                    for ni in range(KT):
                        nc.tensor.matmul(
                            pzq,
                            lhsT=xt_sb[:, t, ni, :],
                            rhs=lmT_sb[:, ni, sl],
                            start=(ni == 0),
                            stop=(ni == KT - 1),
                        )
                    if q < 3:
                        if q % 2 == 0:
                            # even quarters: scalar copy-scale + vector sub
                            nc.scalar.activation(
                                out=ybig[:, t, sl],
                                in_=pzq,
                                func=mybir.ActivationFunctionType.Copy,
                                scale=invs[t],
                            )
                        else:
                            # odd quarters: vector-only chain
                            nc.vector.tensor_scalar(
                                out=ybig[:, t, sl],
                                in0=pzq,
                                scalar1=invs[t],
                                scalar2=None,
                                op0=mybir.AluOpType.mult,
                            )
                        nc.vector.tensor_tensor(
                            out=ybig[:, t, sl],
                            in0=ybig[:, t, sl],
                            in1=t1s[t][:, sl],
                            op=mybir.AluOpType.subtract,
                        )
                        nc.sync.dma_start(out=y_ap[:, t, sl], in_=ybig[:, t, sl])
                    else:
                        # final quarter: compute halves in parallel across
                        # engines, but ship ONE [128,256] DMA - half-DMAs
                        # would be 256B/partition, below the 512B SDMA line
                        # size, and the resulting read-modify-write races
                        # the adjacent half's DMA on the shared HBM line.
                        sla = slice(q * 256, q * 256 + 128)
                        slb = slice(q * 256 + 128, (q + 1) * 256)
                        nc.scalar.activation(
                            out=ybig[:, t, sla],
                            in_=pzq[:, 0:128],
                            func=mybir.ActivationFunctionType.Copy,
                            scale=invs[t],
                        )
                        nc.vector.tensor_scalar(
                            out=ybig[:, t, slb],
                            in0=pzq[:, 128:256],
                            scalar1=invs[t],
                            scalar2=None,
                            op0=mybir.AluOpType.mult,
                        )
                        nc.vector.tensor_tensor(
                            out=ybig[:, t, slb],
                            in0=ybig[:, t, slb],
                            in1=t1s[t][:, slb],
                            op=mybir.AluOpType.subtract,
                        )
                        nc.vector.tensor_tensor(
                            out=ybig[:, t, sla],
                            in0=ybig[:, t, sla],
                            in1=t1s[t][:, sla],
                            op=mybir.AluOpType.subtract,
                        )
                        nc.sync.dma_start(out=y_ap[:, t, sl], in_=ybig[:, t, sl])
            # batched y-out DMAs on the sync queue (idle after input issues)
            if t == 3:
                nc.sync.dma_start(out=y_ap[:, 0:4], in_=ybig[:, 0:4])
            elif t == 7:
                nc.sync.dma_start(out=y_ap[:, 4:8], in_=ybig[:, 4:8])
            elif t == 11:
                nc.sync.dma_start(out=y_ap[:, 8:12], in_=ybig[:, 8:12])
            elif t == 13:
                nc.sync.dma_start(out=y_ap[:, 12:14], in_=ybig[:, 12:14])
            elif t == 14:
                nc.sync.dma_start(out=y_ap[:, 14:15], in_=ybig[:, 14:15])


_NC_CACHE = None


def _get_nc():
    global _NC_CACHE
    if _NC_CACHE is None:
        nc = bacc.Bacc(
            "TRN2", target_bir_lowering=False, debug=False, num_devices=N_CORES
        )
        xt = nc.dram_tensor(
            "xt", [128, NT, KT, 128], mybir.dt.bfloat16, kind="ExternalInput"
        ).ap()
        xrow = nc.dram_tensor(
            "xrow", [128, NT, NIN], mybir.dt.bfloat16, kind="ExternalInput"
        ).ap()
        lmimg = nc.dram_tensor(
            "lmimg", [128, LMW], mybir.dt.bfloat16, kind="ExternalInput"
        ).ap()
        y = nc.dram_tensor(
            "y", [128, NT, P], mybir.dt.bfloat16, kind="ExternalOutput"
        ).ap()
        with tile.TileContext(nc) as tc:
            _build_kernel_body(tc, y, xt, xrow, lmimg)
        nc.compile()
        _NC_CACHE = nc
    return _NC_CACHE


def _in_maps(x: np.ndarray, lm_raw: np.ndarray):
    xb = np.asarray(x, dtype=np.float32).astype(BF16)  # [16384, 1024] bf16
    # packed lm image: per-partition [cb0 | cb1 | lmroll chunks 0..7]
    # (pure layout transform + cast; lmroll = lm_rawT rolled by +16 rows)
    lmr = np.ascontiguousarray(lm_raw, dtype=np.float32).reshape(P, NIN)
    lmroll = np.roll(lmr.T, PAD, axis=0)  # [1024(n'), 1024(p)] f32
    cb = _cb_blocks()  # [2, 128, 128] f32
    lmimg = np.empty((128, LMW), dtype=np.float32)
    lmimg[:, 0:128] = cb[0]
    lmimg[:, 128:256] = cb[1]
    lmimg[:, CBW:] = (
        lmroll.reshape(KT, 128, NIN).transpose(1, 0, 2).reshape(128, KT * NIN)
    )
    lmimg = lmimg.astype(BF16)
    maps = []
    for c in range(N_CORES):
        xs = xb[c * BS : (c + 1) * BS]  # [2048, 1024] bf16
        # xt[nh, t, ni, bh] = x[c*2048 + t*128 + bh, ni*128 + nh]
        xtile = np.ascontiguousarray(
            xs.reshape(NT, 128, KT, 128).transpose(3, 0, 2, 1)
        )  # [128, 16, 8, 128]
        # xrow image: [bh, t, n] = x[c*2048 + t*128 + bh, n]
        xrimg = np.ascontiguousarray(xs.reshape(NT, 128, NIN).transpose(1, 0, 2))
        maps.append({"xt": xtile, "xrow": xrimg, "lmimg": lmimg})
    return maps


def run_spmd(x: np.ndarray, lm_raw: np.ndarray, **kwargs):
    """Run the device kernel; returns (y_full, BassKernelResults)."""
    res = run_bass_kernel_spmd(
        _get_nc(), _in_maps(x, lm_raw), core_ids=list(range(N_CORES)), **kwargs
    )
    # y image per core: [128(bh), 16(t), 1024(p)] -> [2048, 1024]
    y = np.concatenate(
        [r["y"].transpose(1, 0, 2).reshape(BS, P) for r in res.results], axis=0
    )
    return y.reshape(B_FULL, 32, 32).astype(np.float32), res


def kernel(x: np.ndarray, lm_raw: np.ndarray) -> np.ndarray:
    y, _ = run_spmd(x, lm_raw)
    return y
